# revision 35
# baseline (speedup 1.0000x reference)
"""Trainium2 Bass kernel for the BDH dense-transformer problem.

Sharding: data-parallel over B=8 across the 8 NeuronCores (one batch
element per core, no collectives). Each core runs the full 6-layer
network on its [T=2048, D=256] slice.

Precision: all matmuls run single-pass float32r (1 cyc/row on PE for
output free size >= 256; HW-measured effective mantissa ~11 bits,
per-matmul rel err ~1.5e-4). Tensors feeding f32r matmuls are declared
float32r so producer instructions emit f32r-rounded outputs (BIR
verifier requirement); DMA-fed weights stay unrounded fp32 bits, which
HW-measurably matches DVE-rounded operands (the PE rounds internally).
The residual stream vN is kept in full fp32 (f32r-rounding it each
layer dominated end-to-end error); vN_r is a rounded copy consumed only
by the attention accumulate matmul.

Scheduling (engines are in-order; emission order = queue order):
  - LayerNorm statistics batched 4 token-blocks at a time as [128,4]
    ops; variance computed uncentered (E[x^2] - mu^2).
  - cphase v = ln(v + ln(update)) runs almost entirely on the otherwise
    idle Pool engine (second LN needs no mean: v + ln(upd) has exactly
    zero row-mean), emitted inside the MLP's last weight-eighth so it
    overlaps remaining MLP work without delaying ACT/DVE.
  - PE transposes that depend on slow chains are emitted only at points
    where their inputs are already complete (after the q-loop; group 3
    after next-layer attention si=0) so the in-order PE never stalls.
  - attention LN: psA drained PSUM->SBUF with fused row-sums, stats and
    apply on Pool; each si's LN+transposes emitted after the next si's
    PSUM work.
  - rope chunked per 512 t-block on DVE+Pool, emitted as soon as the
    corresponding vT block exists.
"""

import math

import numpy as np
import ml_dtypes

import concourse.bass as bass
import concourse.tile as tile
from concourse import bacc, mybir
from concourse import bass_utils

F32 = mybir.dt.float32
F32R = mybir.dt.float32r
BF16 = mybir.dt.bfloat16
I32 = mybir.dt.int32
ALU = mybir.AluOpType
ACTF = mybir.ActivationFunctionType
AXX = mybir.AxisListType.X

B, T, D, N, H, VOCAB, L = 8, 2048, 256, 8192, 4, 256, 6
EPS = 1e-5
TS = 512          # t-super width
NSUP = T // TS    # 4
NTB = T // 128    # 16
NQ = 8            # weight eighths along N
NCHQ = N // 128 // NQ  # 8 n-chunks per eighth


def build_nc(layers=L):
    nc = bacc.Bacc("TRN2", target_bir_lowering=False, debug=False)

    idx_d = nc.dram_tensor("idxf", [1, T], F32R, kind="ExternalInput")
    wte_d = nc.dram_tensor("wte", [VOCAB, D], F32R, kind="ExternalInput")
    wx_d = nc.dram_tensor("wx", [128, 2, N], F32R, kind="ExternalInput")
    wy_d = nc.dram_tensor("wy", [128, 2, N], F32R, kind="ExternalInput")
    enc_d = nc.dram_tensor("enc", [128, N // 128, D], F32R, kind="ExternalInput")
    ro_d = nc.dram_tensor("ro", [D, VOCAB], F32R, kind="ExternalInput")
    cos_d = nc.dram_tensor("cosT", [128, T], F32, kind="ExternalInput")
    sin_d = nc.dram_tensor("sinT", [128, T], F32, kind="ExternalInput")
    mask_d = nc.dram_tensor("maskbig", [128, 1024], BF16, kind="ExternalInput")
    ident_d = nc.dram_tensor("identm", [128, 128], F32, kind="ExternalInput")
    out_d = nc.dram_tensor("logits", [T, VOCAB], F32, kind="ExternalOutput")

    wx_r, wy_r, enc_r = wx_d.ap(), wy_d.ap(), enc_d.ap()
    wte_r = wte_d.ap().rearrange("(c p) d -> p c d", p=128)
    ro_r = ro_d.ap().rearrange("(c p) d -> p c d", p=128)

    with tile.TileContext(nc) as tc:
        with tc.tile_pool(name="persist", bufs=1) as pp, \
             tc.tile_pool(name="wq", bufs=2) as wq, \
             tc.tile_pool(name="blk", bufs=6) as blkp, \
             tc.tile_pool(name="sc", bufs=12) as scp, \
             tc.tile_pool(name="st", bufs=32) as stp, \
             tc.tile_pool(name="stg", bufs=24) as stgp, \
             tc.tile_pool(name="b4", bufs=4) as b4p, \
             tc.tile_pool(name="ps512", bufs=4, space="PSUM") as ps512, \
             tc.tile_pool(name="ps256", bufs=4, space="PSUM") as ps256:

            vT = [pp.tile([128, T], F32R, name=f"vT{c}", tag=f"vT{c}") for c in range(2)]
            # vN holds the residual stream in full fp32; vN_r is its
            # f32r-rounded copy for the attention accumulate matmul.
            vN = pp.tile([128, NTB, D], F32, name="vN", tag="vN")
            vN_r = pp.tile([128, NTB, D], F32R, name="vN_r", tag="vN_r")
            qrT = [pp.tile([128, T], F32R, name=f"qrT{c}", tag=f"qrT{c}") for c in range(2)]
            lnaT = [pp.tile([128, T], F32R, name=f"lnaT{c}", tag=f"lnaT{c}") for c in range(2)]
            updS = pp.tile([128, NTB, D], F32, name="updS", tag="updS")
            sumsU = pp.tile([128, NTB], F32, name="sumsU", tag="sumsU")

            def updA(tb):
                return updS[:, tb, :]
            cosT = pp.tile([128, T], F32, name="cosT", tag="cosT")
            sinT = pp.tile([128, T], F32, name="sinT", tag="sinT")
            maskb = pp.tile([128, 1024], BF16, name="maskb", tag="maskb")

            ident = pp.tile([128, 128], F32, name="ident", tag="ident")
            iota_f = pp.tile([128, 2], F32, name="iota_f", tag="iota_f")

            nc.sync.dma_start(cosT[:], cos_d.ap())
            nc.sync.dma_start(sinT[:], sin_d.ap())
            nc.sync.dma_start(maskb[:], mask_d.ap())
            nc.sync.dma_start(ident[:], ident_d.ap())

            copy_flip = [0]

            def copy_any(dst, src):
                # alternate PSUM->SBUF copies between ACT and DVE
                # (Pool/GPSIMD cannot access PSUM)
                copy_flip[0] ^= 1
                if copy_flip[0]:
                    nc.scalar.copy(dst, src)
                else:
                    nc.vector.tensor_copy(dst, src)

            def tr128(dst, src):
                pst = ps512.tile([128, 512], F32, name="pst", tag="ps512")
                if src.dtype != F32:
                    src = src.bitcast(F32)
                nc.tensor.transpose(pst[:, :128], src, ident[:])
                copy_any(dst, pst[:, :128])

            def ln_nat(src, dst, sums=None):
                """Single-block LayerNorm (embedding only)."""
                if sums is None:
                    sums = stp.tile([128, 1], F32, name="s1", tag="st")
                    nc.vector.reduce_sum(sums, src, axis=AXX)
                negmean = stp.tile([128, 1], F32, name="negmean", tag="st")
                nc.vector.tensor_scalar_mul(negmean, sums, -1.0 / D)
                sq = scp.tile([128, D], F32, name="sq", tag="sc")
                sqs = stp.tile([128, 1], F32, name="sqs", tag="st")
                nc.scalar.activation(sq, src, ACTF.Square, bias=negmean, scale=1.0,
                                     accum_out=sqs)
                veps = stp.tile([128, 1], F32, name="veps", tag="st")
                nc.vector.tensor_scalar(veps, sqs, 1.0 / D, EPS, op0=ALU.mult, op1=ALU.add)
                sqv = stp.tile([128, 1], F32, name="sqv", tag="st")
                nc.scalar.sqrt(sqv, veps)
                rstd = stp.tile([128, 1], F32, name="rstd", tag="st")
                nc.vector.reciprocal(rstd, sqv)
                negmurs = stp.tile([128, 1], F32, name="negmurs", tag="st")
                nc.vector.tensor_tensor(negmurs, negmean, rstd, op=ALU.mult)
                nc.scalar.activation(dst, src, ACTF.Identity, bias=negmurs, scale=rstd)

            # ---------------- embedding: v = ln(wte[idx]) ----------------
            iota_i = pp.tile([128, 2], I32, name="iota_i", tag="iota_i")
            for c in range(2):
                nc.gpsimd.iota(iota_i[:, c:c + 1], pattern=[[1, 1]], base=c * 128,
                               channel_multiplier=1)
            nc.vector.tensor_copy(iota_f[:], iota_i[:])
            idx_b = lnaT[0]  # scratch alias
            nc.sync.dma_start(idx_b[:], idx_d.ap().partition_broadcast(128))
            for c in range(2):
                # one-hot^T chunk in qrT[c] (scratch alias); 0/1 exact in f32r
                nc.vector.tensor_scalar(qrT[c][:], idx_b[:], iota_f[:, c:c + 1], None,
                                        op0=ALU.is_equal)
            wte_s = blkp.tile([128, 2, D], F32R, name="wte_s", tag="blk")
            nc.sync.dma_start(wte_s[:], wte_r)
            for tb in range(NTB):
                psA = ps256.tile([128, D], F32, name="psE", tag="ps256")
                for c in range(2):
                    nc.tensor.matmul(psA, qrT[c][:, tb * 128:(tb + 1) * 128],
                                     wte_s[:, c, :], start=(c == 0), stop=(c == 1))
                ln_nat(psA, vN[:, tb, :])
                nc.gpsimd.tensor_copy(vN_r[:, tb, :], vN[:, tb, :])
                for c in range(2):
                    tr128(vT[c][:, tb * 128:(tb + 1) * 128], vN[:, tb, c * 128:(c + 1) * 128])

            # ---------------- building blocks ----------------
            rsc = lnaT[1]  # rope scratch (dead region at rope time)

            def rope_chunk(si):
                # qrT[:, si block] = vT*cos +/- rot*sin, split DVE/Pool
                sl = slice(si * TS, (si + 1) * TS)
                e0 = nc.vector if si % 2 == 0 else nc.gpsimd
                e1 = nc.gpsimd if si % 2 == 0 else nc.vector
                e0.tensor_tensor(qrT[0][:, sl], vT[0][:, sl], cosT[:, sl], op=ALU.mult)
                e1.tensor_tensor(rsc[:, sl], vT[1][:, sl], sinT[:, sl], op=ALU.mult)
                e0.tensor_tensor(qrT[0][:, sl], qrT[0][:, sl], rsc[:, sl], op=ALU.subtract)
                e1.tensor_tensor(qrT[1][:, sl], vT[1][:, sl], cosT[:, sl], op=ALU.mult)
                e0.tensor_tensor(rsc[:, sl], vT[0][:, sl], sinT[:, sl], op=ALU.mult)
                e1.tensor_tensor(qrT[1][:, sl], qrT[1][:, sl], rsc[:, sl], op=ALU.add)

            att_state = {}

            def att_psum(si):
                # energy blocks + psA accumulation for one si super-block
                psA = [ps256.tile([128, D], F32, name="psA", tag="ps256")
                       for _ in range(4)]

                def psa_emit(eT, sc):
                    for tb4 in range(4):
                        tb = si * 4 + tb4
                        if sc <= tb:
                            nc.tensor.matmul(psA[tb4],
                                             eT[:, tb4 * 128:(tb4 + 1) * 128],
                                             vN_r[:, sc, :], start=(sc == 0),
                                             stop=(sc == tb))
                pend = None
                for sc in range(4 * si + 4):
                    k = sc - 4 * si
                    off = max(k, 0) * 128  # masked-out leading cols skipped
                    psE = ps512.tile([128, TS], F32, name="psE", tag="ps512")
                    for c in range(2):
                        nc.tensor.matmul(psE[:, off:],
                                         qrT[c][:, sc * 128:(sc + 1) * 128],
                                         qrT[c][:, si * TS + off:(si + 1) * TS],
                                         start=(c == 0), stop=(c == 1))
                    eT = blkp.tile([128, TS], F32R, name="eT", tag="blk")
                    if k < 0:
                        nc.scalar.copy(eT[:], psE[:])
                    else:
                        nc.vector.tensor_tensor(
                            eT[:, off:], psE[:, off:], maskb[:, 384: 896 - off],
                            op=ALU.mult)
                    # software pipeline: psA for the PREVIOUS sc, so PE isn't
                    # waiting on this sc's eT
                    if pend is not None:
                        psa_emit(*pend)
                    pend = (eT, sc)
                psa_emit(*pend)
                att_state[si] = psA

            def att_drain(si):
                # drain psA to SBUF with fused row-sums (ACT/DVE). Emitted
                # immediately after att_psum(si) so the PSUM banks recycle
                # with correct dependencies before the next si allocates them.
                psAb = att_state.pop(si)
                psAs = b4p.tile([128, 4, D], F32, name="psAs", tag="b4")
                sums = stgp.tile([128, 4], F32, name="at_sums", tag="stg")
                for j in range(4):
                    src = psAb[j][:]
                    if j % 2 == 0:
                        nc.scalar.activation(psAs[:, j, :], src, ACTF.Identity,
                                             accum_out=sums[:, j:j + 1])
                    else:
                        nc.vector.tensor_scalar(psAs[:, j, :], src, 0.0, 0.0,
                                                op0=ALU.add, op1=ALU.add,
                                                accum_out=sums[:, j:j + 1])
                att_state[(si, 'drained')] = (psAs, sums)

            def att_ln_rest(si):
                # stats on ACT/Pool, apply via Pool broadcast-tt, transposes
                psAs, sums = att_state.pop((si, 'drained'))
                sqs = stgp.tile([128, 4], F32, name="at_sqs", tag="stg")
                sqf = scp.tile([128, D], F32, name="at_scr", tag="sc")
                for j in range(4):
                    nc.scalar.activation(sqf, psAs[:, j, :], ACTF.Square,
                                         accum_out=sqs[:, j:j + 1])
                negmean = stgp.tile([128, 4], F32, name="at_nm", tag="stg")
                nc.gpsimd.tensor_scalar(negmean, sums, -1.0 / D, 0.0,
                                        op0=ALU.mult, op1=ALU.add)
                msq = stgp.tile([128, 4], F32, name="at_msq", tag="stg")
                nc.gpsimd.tensor_tensor(msq, negmean, negmean, op=ALU.mult)
                ex2 = stgp.tile([128, 4], F32, name="at_ex2", tag="stg")
                nc.gpsimd.tensor_scalar(ex2, sqs, 1.0 / D, EPS, op0=ALU.mult,
                                        op1=ALU.add)
                veps = stgp.tile([128, 4], F32, name="at_veps", tag="stg")
                nc.gpsimd.tensor_tensor(veps, ex2, msq, op=ALU.subtract)
                sqv = stgp.tile([128, 4], F32, name="at_sqv", tag="stg")
                nc.scalar.sqrt(sqv, veps)
                rstd = stgp.tile([128, 4], F32, name="at_rstd", tag="stg")
                nc.vector.reciprocal(rstd, sqv)
                lnas = []
                for j in range(4):
                    lna = scp.tile([128, D], F32, name="lna_n", tag="sc")
                    nc.gpsimd.tensor_tensor(
                        lna, psAs[:, j, :],
                        negmean[:, j:j + 1].broadcast_to((128, D)), op=ALU.add)
                    nc.gpsimd.tensor_tensor(
                        lna, lna, rstd[:, j:j + 1].broadcast_to((128, D)),
                        op=ALU.mult)
                    lnas.append(lna)
                for tb4 in range(4):
                    tb = si * 4 + tb4
                    for c in range(2):
                        tr128(lnaT[c][:, tb * 128:(tb + 1) * 128],
                              lnas[tb4][:, c * 128:(c + 1) * 128])

            def cphase_chain(g):
                # v = ln(v + ln(update)) values for token-blocks 4g..4g+3,
                # entirely on Pool (plus tiny ACT sqrt / DVE reciprocal).
                # Uncentered variance; the second LN exploits that
                # v + ln(upd) has exactly zero row-mean.
                tbs = list(range(4 * g, 4 * g + 4))
                sqs = stgp.tile([128, 4], F32, name="cp_sqs", tag="stg")
                sqf = scp.tile([128, D], F32, name="cp_scr", tag="sc")
                for i, tb in enumerate(tbs):
                    nc.scalar.activation(sqf, updA(tb), ACTF.Square,
                                         accum_out=sqs[:, i:i + 1])
                negmean = stgp.tile([128, 4], F32, name="cp_nm", tag="stg")
                nc.gpsimd.tensor_scalar(negmean, sumsU[:, 4 * g:4 * g + 4],
                                        -1.0 / D, 0.0, op0=ALU.mult, op1=ALU.add)
                msq = stgp.tile([128, 4], F32, name="cp_msq", tag="stg")
                nc.gpsimd.tensor_tensor(msq, negmean, negmean, op=ALU.mult)
                ex2 = stgp.tile([128, 4], F32, name="cp_ex2", tag="stg")
                nc.gpsimd.tensor_scalar(ex2, sqs, 1.0 / D, EPS, op0=ALU.mult,
                                        op1=ALU.add)
                veps = stgp.tile([128, 4], F32, name="cp_veps", tag="stg")
                nc.gpsimd.tensor_tensor(veps, ex2, msq, op=ALU.subtract)
                sqv = stgp.tile([128, 4], F32, name="cp_sqv", tag="stg")
                nc.scalar.sqrt(sqv, veps)
                rstd = stgp.tile([128, 4], F32, name="cp_rstd", tag="stg")
                nc.vector.reciprocal(rstd, sqv)
                sq2s = stgp.tile([128, 4], F32, name="cp_sq2s", tag="stg")
                vmids = []
                for i, tb in enumerate(tbs):
                    lnu = scp.tile([128, D], F32, name="lnu", tag="sc")
                    nc.gpsimd.tensor_tensor(
                        lnu, updA(tb),
                        negmean[:, i:i + 1].broadcast_to((128, D)), op=ALU.add)
                    nc.gpsimd.tensor_tensor(
                        lnu, lnu, rstd[:, i:i + 1].broadcast_to((128, D)),
                        op=ALU.mult)
                    vmid = scp.tile([128, D], F32, name="vmid", tag="sc")
                    nc.gpsimd.tensor_tensor(vmid, lnu, vN[:, tb, :], op=ALU.add)
                    nc.scalar.activation(sqf, vmid, ACTF.Square,
                                         accum_out=sq2s[:, i:i + 1])
                    vmids.append(vmid)
                veps2 = stgp.tile([128, 4], F32, name="cp_veps2", tag="stg")
                nc.gpsimd.tensor_scalar(veps2, sq2s, 1.0 / D, EPS, op0=ALU.mult,
                                        op1=ALU.add)
                sqv2 = stgp.tile([128, 4], F32, name="cp_sqv2", tag="stg")
                nc.scalar.sqrt(sqv2, veps2)
                rstd2 = stgp.tile([128, 4], F32, name="cp_rstd2", tag="stg")
                nc.vector.reciprocal(rstd2, sqv2)
                for i, tb in enumerate(tbs):
                    nc.gpsimd.tensor_tensor(
                        vN[:, tb, :], vmids[i],
                        rstd2[:, i:i + 1].broadcast_to((128, D)), op=ALU.mult)
                    nc.gpsimd.tensor_copy(vN_r[:, tb, :], vN[:, tb, :])

            def cphase_tr(g):
                # vT transposes for group g (emitted where inputs are ready)
                for tb in range(4 * g, 4 * g + 4):
                    for c in range(2):
                        tr128(vT[c][:, tb * 128:(tb + 1) * 128],
                              vN[:, tb, c * 128:(c + 1) * 128])

            def mlp(layer):
                # streamed over 8 weight-eighths; cphase chains emitted inside
                # the last eighth as each si group's update completes
                for q in range(NQ):
                    qs = slice(q * (N // NQ), (q + 1) * (N // NQ))
                    wxq = wq.tile([128, 2, N // NQ], F32R, name="wxq", tag="wxq")
                    nc.sync.dma_start(wxq[:], wx_r[:, :, qs])
                    wyq = wq.tile([128, 2, N // NQ], F32R, name="wyq", tag="wyq")
                    nc.sync.dma_start(wyq[:], wy_r[:, :, qs])
                    encq = wq.tile([128, NCHQ, D], F32R, name="encq", tag="encq")
                    nc.sync.dma_start(encq[:], enc_r[:, q * NCHQ:(q + 1) * NCHQ, :])
                    for si in range(NSUP):
                        sl = slice(si * TS, (si + 1) * TS)
                        psU = [ps256.tile([128, D], F32, name="psU", tag="ps256")
                               for _ in range(4)]
                        def psu_emit(ysb, nch):
                            for tb4 in range(4):
                                t4 = slice(tb4 * 128, (tb4 + 1) * 128)
                                nc.tensor.matmul(
                                    psU[tb4], ysb[:, t4], encq[:, nch, :],
                                    start=(nch == 0), stop=(nch == NCHQ - 1))
                        pend = None
                        for nch in range(NCHQ):
                            psX = ps512.tile([128, TS], F32, name="psX", tag="ps512")
                            psY = ps512.tile([128, TS], F32, name="psY", tag="ps512")
                            ns = slice(nch * 128, (nch + 1) * 128)
                            for i, (wt, act) in enumerate(((wxq, vT), (wyq, lnaT))):
                                ps = psX if i == 0 else psY
                                for c in range(2):
                                    nc.tensor.matmul(ps, wt[:, c, ns], act[c][:, sl],
                                                     start=(c == 0), stop=(c == 1))
                            xr = blkp.tile([128, TS], F32, name="xr", tag="blk")
                            nc.scalar.activation(xr, psX, ACTF.Relu)
                            ysb = blkp.tile([128, TS], F32R, name="ysb", tag="blk")
                            nc.vector.scalar_tensor_tensor(
                                ysb, psY, 0.0, xr, op0=ALU.max, op1=ALU.mult)
                            # software pipeline: psU for the PREVIOUS nch, so
                            # PE isn't waiting on this nch's ysb
                            if pend is not None:
                                psu_emit(*pend)
                            pend = (ysb, nch)
                        psu_emit(*pend)
                        if q == 0:
                            for tb4 in range(4):
                                copy_any(updA(si * 4 + tb4), psU[tb4])
                        elif q < NQ - 1:
                            # drain PSUM fast (ACT/DVE), accumulate on Pool
                            psUs = b4p.tile([128, 4, D], F32, name="psUs", tag="b4")
                            for tb4 in range(4):
                                copy_any(psUs[:, tb4, :], psU[tb4])
                            for tb4 in range(4):
                                tb = si * 4 + tb4
                                nc.gpsimd.tensor_tensor(updA(tb), psUs[:, tb4, :],
                                                        updA(tb), op=ALU.add)
                        else:
                            for tb4 in range(4):
                                tb = si * 4 + tb4
                                nc.vector.scalar_tensor_tensor(
                                    updA(tb), psU[tb4], 0.0, updA(tb), op0=ALU.add,
                                    op1=ALU.add, accum_out=sumsU[:, tb:tb + 1])
                        if q == NQ - 1:
                            cphase_chain(si)

            # ---------------- program ----------------
            for si in range(NSUP):
                rope_chunk(si)
            # attention staggered: drain right after each si (correct PSUM
            # recycling); stats/apply/transposes overlap the next si
            att_psum(0)
            att_drain(0)
            att_psum(1)
            att_drain(1)
            att_ln_rest(0)
            att_psum(2)
            att_drain(2)
            att_ln_rest(1)
            att_psum(3)
            att_drain(3)
            att_ln_rest(2)
            att_ln_rest(3)

            for layer in range(layers):
                mlp(layer)
                if layer < layers - 1:
                    cphase_tr(0)
                    rope_chunk(0)
                    cphase_tr(1)
                    rope_chunk(1)
                    att_psum(0)
                    att_drain(0)
                    cphase_tr(2)
                    rope_chunk(2)
                    cphase_tr(3)
                    rope_chunk(3)
                    att_psum(1)
                    att_drain(1)
                    att_ln_rest(0)
                    att_psum(2)
                    att_drain(2)
                    att_ln_rest(1)
                    att_psum(3)
                    att_drain(3)
                    att_ln_rest(2)
                    att_ln_rest(3)
                else:
                    for g in range(4):
                        cphase_tr(g)

            # ---------------- readout ----------------
            ro_s = blkp.tile([128, 2, D], F32R, name="ro_s", tag="blk")
            nc.sync.dma_start(ro_s[:], ro_r)
            for tb in range(NTB):
                psR = ps256.tile([128, D], F32, name="psR", tag="ps256")
                for c in range(2):
                    nc.tensor.matmul(psR, vT[c][:, tb * 128:(tb + 1) * 128],
                                     ro_s[:, c, :], start=(c == 0), stop=(c == 1))
                lo = scp.tile([128, VOCAB], F32, name="lo", tag="sc")
                copy_any(lo[:], psR[:])
                nc.sync.dma_start(out_d.ap()[tb * 128:(tb + 1) * 128, :], lo[:])

    nc.compile()
    return nc


_NC_CACHE = {}


def get_nc():
    if "nc" not in _NC_CACHE:
        _NC_CACHE["nc"] = build_nc()
    return _NC_CACHE["nc"]


def make_host_inputs(idx, wte, encoder, decoder_x, decoder_y, readout):
    idx = np.asarray(idx)
    wte = np.asarray(wte, dtype=np.float32)
    encoder = np.asarray(encoder, dtype=np.float32)
    decoder_x = np.asarray(decoder_x, dtype=np.float32)
    decoder_y = np.asarray(decoder_y, dtype=np.float32)
    readout = np.asarray(readout, dtype=np.float32)

    wx = decoder_x.transpose(1, 0, 2).reshape(D, N)
    wy = decoder_y.transpose(1, 0, 2).reshape(D, N)
    # partition-contiguous layouts for fast DMA: [p, c, n] with d = c*128 + p
    wx = np.ascontiguousarray(wx.reshape(2, 128, N).transpose(1, 0, 2))
    wy = np.ascontiguousarray(wy.reshape(2, 128, N).transpose(1, 0, 2))
    # enc: [p, o, d] with n = o*128 + p
    enc_s = np.ascontiguousarray(encoder.reshape(N // 128, 128, D).transpose(1, 0, 2))

    com = {"wx": wx, "wy": wy, "enc": enc_s}

    inv_freq = 1.0 / (10000.0 ** (np.arange(0, D, 2, dtype=np.float32) / D))  # [128]
    t = np.arange(T, dtype=np.float32)
    freqsT = inv_freq[:, None] * t[None, :]                   # [128, T]
    com["cosT"] = np.cos(freqsT).astype(np.float32)
    com["sinT"] = np.sin(freqsT).astype(np.float32)

    s_idx = np.arange(128, dtype=np.int32)[:, None]
    c_idx = np.arange(1024, dtype=np.int32)[None, :]
    com["maskbig"] = (s_idx <= c_idx - 384).astype(ml_dtypes.bfloat16)
    com["wte"] = wte
    com["ro"] = readout
    com["identm"] = np.eye(128, dtype=np.float32)

    in_maps = []
    for b in range(B):
        m = dict(com)
        m["idxf"] = idx[b].astype(np.float32).reshape(1, T)
        in_maps.append(m)
    return in_maps


def kernel(idx, wte, encoder, decoder_x, decoder_y, readout):
    nc = get_nc()
    in_maps = make_host_inputs(idx, wte, encoder, decoder_x, decoder_y, readout)
    res = bass_utils.run_bass_kernel_spmd(nc, in_maps, core_ids=list(range(B)))
    out = np.stack([res.results[b]["logits"] for b in range(B)], axis=0)
    return out.astype(np.float32)


# revision 37
# speedup vs baseline: 1.0033x; 1.0033x over previous
"""Trainium2 Bass kernel for the BDH dense-transformer problem.

Sharding: data-parallel over B=8 across the 8 NeuronCores (one batch
element per core, no collectives). Each core runs the full 6-layer
network on its [T=2048, D=256] slice.

Precision: all matmuls run single-pass float32r (1 cyc/row on PE for
output free size >= 256; HW-measured effective mantissa ~11 bits,
per-matmul rel err ~1.5e-4; 3-4x faster than fp32 / bf16x2-split
3-pass). Tensors feeding f32r matmuls are declared float32r so producer
instructions emit f32r-rounded outputs (BIR verifier requirement);
DMA-fed weights stay unrounded fp32 bits, which HW-measurably matches
DVE-rounded operands (the PE rounds operands internally). The residual
stream vN is kept in full fp32 -- f32r-rounding it every layer was the
dominant end-to-end error term (1.3e-2 -> 3.4e-3) -- with vN_r a
rounded copy consumed only by the attention accumulate matmul; the
update accumulator updS is dedicated fp32 (no partial-sum rounding).

Structure:
  - token embedding via one-hot matmul (iota + is_equal + PE)
  - v kept in both layouts: vT [D,T] (f32r) and vN [T,D] (fp32)
  - causal linear attention block-wise: energyT = qr@qr^T per
    [s128, t512] block (PSUM, diagonal blocks skip masked-out leading
    cols), bf16-mask multiply, aN accumulated in PSUM over s-chunks
  - MLP streamed over N in eighths (weights DMA'd per layer in
    host-pre-shuffled partition-contiguous layouts), relu(x)*relu(y)
    fused via ACT relu + DVE scalar_tensor_tensor, update accumulated
    in PSUM per eighth then drained into updS

Scheduling (engines are in-order; emission order = per-engine queue
order, so placement of every instruction matters):
  - one PSUM accumulation group per 2KB bank (start=True zeroes the
    whole bank): psU/psA get 4 banks (ps256), psX/psY/psE/pst 4 (ps512)
  - LayerNorm statistics batched 4 token-blocks at a time as [128,4]
    ops; variance computed uncentered (E[x^2] - mu^2), row sums fused
    into drain/accumulate ops (accum_out)
  - cphase v = ln(v + ln(update)) runs almost entirely on the otherwise
    idle Pool engine (applies via stride-0 broadcast tensor_tensor; the
    second LN needs no mean handling since v + ln(upd) has exactly zero
    row-mean), emitted inside the MLP's last weight-eighth so it
    overlaps remaining MLP work without delaying ACT/DVE (Pool cannot
    read PSUM or use pointer-scalar ops, hence the drain dance)
  - psU drained PSUM->SBUF via alternating ACT/DVE copies, accumulated
    into updS on Pool; frees banks fast at si transitions
  - attention LN: psA drained with fused row-sums right after each si
    (correct PSUM recycling), stats/apply on Pool, lnaT transposes
    emitted after the NEXT si's PSUM work so the in-order PE never
    stalls on them
  - vT transposes emitted after the whole MLP (inputs complete);
    group 2/3 behind next-layer attention si=0
  - rope chunked per 512 t-block on DVE+Pool
"""

import math

import numpy as np
import ml_dtypes

import concourse.bass as bass
import concourse.tile as tile
from concourse import bacc, mybir
from concourse import bass_utils

F32 = mybir.dt.float32
F32R = mybir.dt.float32r
BF16 = mybir.dt.bfloat16
I32 = mybir.dt.int32
ALU = mybir.AluOpType
ACTF = mybir.ActivationFunctionType
AXX = mybir.AxisListType.X

B, T, D, N, H, VOCAB, L = 8, 2048, 256, 8192, 4, 256, 6
EPS = 1e-5
TS = 512          # t-super width
NSUP = T // TS    # 4
NTB = T // 128    # 16
NQ = 8            # weight eighths along N
NCHQ = N // 128 // NQ  # 8 n-chunks per eighth


def build_nc(layers=L):
    nc = bacc.Bacc("TRN2", target_bir_lowering=False, debug=False)

    idx_d = nc.dram_tensor("idxf", [1, T], F32R, kind="ExternalInput")
    wte_d = nc.dram_tensor("wte", [VOCAB, D], F32R, kind="ExternalInput")
    wx_d = nc.dram_tensor("wx", [128, 2, N], F32R, kind="ExternalInput")
    wy_d = nc.dram_tensor("wy", [128, 2, N], F32R, kind="ExternalInput")
    enc_d = nc.dram_tensor("enc", [128, N // 128, D], F32R, kind="ExternalInput")
    ro_d = nc.dram_tensor("ro", [D, VOCAB], F32R, kind="ExternalInput")
    cos_d = nc.dram_tensor("cosT", [128, T], F32, kind="ExternalInput")
    sin_d = nc.dram_tensor("sinT", [128, T], F32, kind="ExternalInput")
    mask_d = nc.dram_tensor("maskbig", [128, 1024], BF16, kind="ExternalInput")
    ident_d = nc.dram_tensor("identm", [128, 128], F32, kind="ExternalInput")
    out_d = nc.dram_tensor("logits", [T, VOCAB], F32, kind="ExternalOutput")

    wx_r, wy_r, enc_r = wx_d.ap(), wy_d.ap(), enc_d.ap()
    wte_r = wte_d.ap().rearrange("(c p) d -> p c d", p=128)
    ro_r = ro_d.ap().rearrange("(c p) d -> p c d", p=128)

    with tile.TileContext(nc) as tc:
        with tc.tile_pool(name="persist", bufs=1) as pp, \
             tc.tile_pool(name="wq", bufs=2) as wq, \
             tc.tile_pool(name="blk", bufs=6) as blkp, \
             tc.tile_pool(name="sc", bufs=12) as scp, \
             tc.tile_pool(name="st", bufs=32) as stp, \
             tc.tile_pool(name="stg", bufs=24) as stgp, \
             tc.tile_pool(name="b4", bufs=4) as b4p, \
             tc.tile_pool(name="ps512", bufs=4, space="PSUM") as ps512, \
             tc.tile_pool(name="ps256", bufs=4, space="PSUM") as ps256:

            vT = [pp.tile([128, T], F32R, name=f"vT{c}", tag=f"vT{c}") for c in range(2)]
            # vN holds the residual stream in full fp32; vN_r is its
            # f32r-rounded copy for the attention accumulate matmul.
            vN = pp.tile([128, NTB, D], F32, name="vN", tag="vN")
            vN_r = pp.tile([128, NTB, D], F32R, name="vN_r", tag="vN_r")
            qrT = [pp.tile([128, T], F32R, name=f"qrT{c}", tag=f"qrT{c}") for c in range(2)]
            lnaT = [pp.tile([128, T], F32R, name=f"lnaT{c}", tag=f"lnaT{c}") for c in range(2)]
            updS = pp.tile([128, NTB, D], F32, name="updS", tag="updS")
            sumsU = pp.tile([128, NTB], F32, name="sumsU", tag="sumsU")

            def updA(tb):
                return updS[:, tb, :]
            cosT = pp.tile([128, T], F32, name="cosT", tag="cosT")
            sinT = pp.tile([128, T], F32, name="sinT", tag="sinT")
            maskb = pp.tile([128, 1024], BF16, name="maskb", tag="maskb")

            ident = pp.tile([128, 128], F32, name="ident", tag="ident")
            iota_f = pp.tile([128, 2], F32, name="iota_f", tag="iota_f")

            nc.sync.dma_start(cosT[:], cos_d.ap())
            nc.sync.dma_start(sinT[:], sin_d.ap())
            nc.sync.dma_start(maskb[:], mask_d.ap())
            nc.sync.dma_start(ident[:], ident_d.ap())

            copy_flip = [0]

            def copy_any(dst, src):
                # alternate PSUM->SBUF copies between ACT and DVE
                # (Pool/GPSIMD cannot access PSUM)
                copy_flip[0] ^= 1
                if copy_flip[0]:
                    nc.scalar.copy(dst, src)
                else:
                    nc.vector.tensor_copy(dst, src)

            def tr128(dst, src):
                pst = ps512.tile([128, 512], F32, name="pst", tag="ps512")
                if src.dtype != F32:
                    src = src.bitcast(F32)
                nc.tensor.transpose(pst[:, :128], src, ident[:])
                copy_any(dst, pst[:, :128])

            def ln_nat(src, dst, sums=None):
                """Single-block LayerNorm (embedding only)."""
                if sums is None:
                    sums = stp.tile([128, 1], F32, name="s1", tag="st")
                    nc.vector.reduce_sum(sums, src, axis=AXX)
                negmean = stp.tile([128, 1], F32, name="negmean", tag="st")
                nc.vector.tensor_scalar_mul(negmean, sums, -1.0 / D)
                sq = scp.tile([128, D], F32, name="sq", tag="sc")
                sqs = stp.tile([128, 1], F32, name="sqs", tag="st")
                nc.scalar.activation(sq, src, ACTF.Square, bias=negmean, scale=1.0,
                                     accum_out=sqs)
                veps = stp.tile([128, 1], F32, name="veps", tag="st")
                nc.vector.tensor_scalar(veps, sqs, 1.0 / D, EPS, op0=ALU.mult, op1=ALU.add)
                sqv = stp.tile([128, 1], F32, name="sqv", tag="st")
                nc.scalar.sqrt(sqv, veps)
                rstd = stp.tile([128, 1], F32, name="rstd", tag="st")
                nc.vector.reciprocal(rstd, sqv)
                negmurs = stp.tile([128, 1], F32, name="negmurs", tag="st")
                nc.vector.tensor_tensor(negmurs, negmean, rstd, op=ALU.mult)
                nc.scalar.activation(dst, src, ACTF.Identity, bias=negmurs, scale=rstd)

            # ---------------- embedding: v = ln(wte[idx]) ----------------
            iota_i = pp.tile([128, 2], I32, name="iota_i", tag="iota_i")
            for c in range(2):
                nc.gpsimd.iota(iota_i[:, c:c + 1], pattern=[[1, 1]], base=c * 128,
                               channel_multiplier=1)
            nc.vector.tensor_copy(iota_f[:], iota_i[:])
            idx_b = lnaT[0]  # scratch alias
            nc.sync.dma_start(idx_b[:], idx_d.ap().partition_broadcast(128))
            for c in range(2):
                # one-hot^T chunk in qrT[c] (scratch alias); 0/1 exact in f32r
                nc.vector.tensor_scalar(qrT[c][:], idx_b[:], iota_f[:, c:c + 1], None,
                                        op0=ALU.is_equal)
            wte_s = blkp.tile([128, 2, D], F32R, name="wte_s", tag="blk")
            nc.sync.dma_start(wte_s[:], wte_r)
            for tb in range(NTB):
                psA = ps256.tile([128, D], F32, name="psE", tag="ps256")
                for c in range(2):
                    nc.tensor.matmul(psA, qrT[c][:, tb * 128:(tb + 1) * 128],
                                     wte_s[:, c, :], start=(c == 0), stop=(c == 1))
                ln_nat(psA, vN[:, tb, :])
                nc.gpsimd.tensor_copy(vN_r[:, tb, :], vN[:, tb, :])
                for c in range(2):
                    tr128(vT[c][:, tb * 128:(tb + 1) * 128], vN[:, tb, c * 128:(c + 1) * 128])

            # ---------------- building blocks ----------------
            rsc = lnaT[1]  # rope scratch (dead region at rope time)

            def rope_chunk(si):
                # qrT[:, si block] = vT*cos +/- rot*sin, split DVE/Pool
                sl = slice(si * TS, (si + 1) * TS)
                e0 = nc.vector if si % 2 == 0 else nc.gpsimd
                e1 = nc.gpsimd if si % 2 == 0 else nc.vector
                e0.tensor_tensor(qrT[0][:, sl], vT[0][:, sl], cosT[:, sl], op=ALU.mult)
                e1.tensor_tensor(rsc[:, sl], vT[1][:, sl], sinT[:, sl], op=ALU.mult)
                e0.tensor_tensor(qrT[0][:, sl], qrT[0][:, sl], rsc[:, sl], op=ALU.subtract)
                e1.tensor_tensor(qrT[1][:, sl], vT[1][:, sl], cosT[:, sl], op=ALU.mult)
                e0.tensor_tensor(rsc[:, sl], vT[0][:, sl], sinT[:, sl], op=ALU.mult)
                e1.tensor_tensor(qrT[1][:, sl], qrT[1][:, sl], rsc[:, sl], op=ALU.add)

            att_state = {}

            def att_psum(si):
                # energy blocks + psA accumulation for one si super-block
                psA = [ps256.tile([128, D], F32, name="psA", tag="ps256")
                       for _ in range(4)]

                def psa_emit(eT, sc):
                    for tb4 in range(4):
                        tb = si * 4 + tb4
                        if sc <= tb:
                            nc.tensor.matmul(psA[tb4],
                                             eT[:, tb4 * 128:(tb4 + 1) * 128],
                                             vN_r[:, sc, :], start=(sc == 0),
                                             stop=(sc == tb))
                for sc in range(4 * si + 4):
                    k = sc - 4 * si
                    off = max(k, 0) * 128  # masked-out leading cols skipped
                    psE = ps512.tile([128, TS], F32, name="psE", tag="ps512")
                    for c in range(2):
                        nc.tensor.matmul(psE[:, off:],
                                         qrT[c][:, sc * 128:(sc + 1) * 128],
                                         qrT[c][:, si * TS + off:(si + 1) * TS],
                                         start=(c == 0), stop=(c == 1))
                    eT = blkp.tile([128, TS], F32R, name="eT", tag="blk")
                    if k < 0:
                        nc.scalar.copy(eT[:], psE[:])
                    else:
                        nc.vector.tensor_tensor(
                            eT[:, off:], psE[:, off:], maskb[:, 384: 896 - off],
                            op=ALU.mult)
                    psa_emit(eT, sc)
                att_state[si] = psA

            def att_drain(si):
                # drain psA to SBUF with fused row-sums (ACT/DVE). Emitted
                # immediately after att_psum(si) so the PSUM banks recycle
                # with correct dependencies before the next si allocates them.
                psAb = att_state.pop(si)
                psAs = b4p.tile([128, 4, D], F32, name="psAs", tag="b4")
                sums = stgp.tile([128, 4], F32, name="at_sums", tag="stg")
                for j in range(4):
                    src = psAb[j][:]
                    if j % 2 == 0:
                        nc.scalar.activation(psAs[:, j, :], src, ACTF.Identity,
                                             accum_out=sums[:, j:j + 1])
                    else:
                        nc.vector.tensor_scalar(psAs[:, j, :], src, 0.0, 0.0,
                                                op0=ALU.add, op1=ALU.add,
                                                accum_out=sums[:, j:j + 1])
                att_state[(si, 'drained')] = (psAs, sums)

            def att_ln_rest(si):
                # stats on ACT/Pool, apply via Pool broadcast-tt, transposes
                psAs, sums = att_state.pop((si, 'drained'))
                sqs = stgp.tile([128, 4], F32, name="at_sqs", tag="stg")
                sqf = scp.tile([128, D], F32, name="at_scr", tag="sc")
                for j in range(4):
                    nc.scalar.activation(sqf, psAs[:, j, :], ACTF.Square,
                                         accum_out=sqs[:, j:j + 1])
                negmean = stgp.tile([128, 4], F32, name="at_nm", tag="stg")
                nc.gpsimd.tensor_scalar(negmean, sums, -1.0 / D, 0.0,
                                        op0=ALU.mult, op1=ALU.add)
                msq = stgp.tile([128, 4], F32, name="at_msq", tag="stg")
                nc.gpsimd.tensor_tensor(msq, negmean, negmean, op=ALU.mult)
                ex2 = stgp.tile([128, 4], F32, name="at_ex2", tag="stg")
                nc.gpsimd.tensor_scalar(ex2, sqs, 1.0 / D, EPS, op0=ALU.mult,
                                        op1=ALU.add)
                veps = stgp.tile([128, 4], F32, name="at_veps", tag="stg")
                nc.gpsimd.tensor_tensor(veps, ex2, msq, op=ALU.subtract)
                sqv = stgp.tile([128, 4], F32, name="at_sqv", tag="stg")
                nc.scalar.sqrt(sqv, veps)
                rstd = stgp.tile([128, 4], F32, name="at_rstd", tag="stg")
                nc.vector.reciprocal(rstd, sqv)
                lnas = []
                for j in range(4):
                    lna = scp.tile([128, D], F32, name="lna_n", tag="sc")
                    nc.gpsimd.tensor_tensor(
                        lna, psAs[:, j, :],
                        negmean[:, j:j + 1].broadcast_to((128, D)), op=ALU.add)
                    nc.gpsimd.tensor_tensor(
                        lna, lna, rstd[:, j:j + 1].broadcast_to((128, D)),
                        op=ALU.mult)
                    lnas.append(lna)
                for tb4 in range(4):
                    tb = si * 4 + tb4
                    for c in range(2):
                        tr128(lnaT[c][:, tb * 128:(tb + 1) * 128],
                              lnas[tb4][:, c * 128:(c + 1) * 128])

            def cphase_chain(g):
                # v = ln(v + ln(update)) values for token-blocks 4g..4g+3,
                # entirely on Pool (plus tiny ACT sqrt / DVE reciprocal).
                # Uncentered variance; the second LN exploits that
                # v + ln(upd) has exactly zero row-mean.
                tbs = list(range(4 * g, 4 * g + 4))
                sqs = stgp.tile([128, 4], F32, name="cp_sqs", tag="stg")
                sqf = scp.tile([128, D], F32, name="cp_scr", tag="sc")
                for i, tb in enumerate(tbs):
                    nc.scalar.activation(sqf, updA(tb), ACTF.Square,
                                         accum_out=sqs[:, i:i + 1])
                negmean = stgp.tile([128, 4], F32, name="cp_nm", tag="stg")
                nc.gpsimd.tensor_scalar(negmean, sumsU[:, 4 * g:4 * g + 4],
                                        -1.0 / D, 0.0, op0=ALU.mult, op1=ALU.add)
                msq = stgp.tile([128, 4], F32, name="cp_msq", tag="stg")
                nc.gpsimd.tensor_tensor(msq, negmean, negmean, op=ALU.mult)
                ex2 = stgp.tile([128, 4], F32, name="cp_ex2", tag="stg")
                nc.gpsimd.tensor_scalar(ex2, sqs, 1.0 / D, EPS, op0=ALU.mult,
                                        op1=ALU.add)
                veps = stgp.tile([128, 4], F32, name="cp_veps", tag="stg")
                nc.gpsimd.tensor_tensor(veps, ex2, msq, op=ALU.subtract)
                sqv = stgp.tile([128, 4], F32, name="cp_sqv", tag="stg")
                nc.scalar.sqrt(sqv, veps)
                rstd = stgp.tile([128, 4], F32, name="cp_rstd", tag="stg")
                nc.vector.reciprocal(rstd, sqv)
                sq2s = stgp.tile([128, 4], F32, name="cp_sq2s", tag="stg")
                vmids = []
                for i, tb in enumerate(tbs):
                    lnu = scp.tile([128, D], F32, name="lnu", tag="sc")
                    nc.gpsimd.tensor_tensor(
                        lnu, updA(tb),
                        negmean[:, i:i + 1].broadcast_to((128, D)), op=ALU.add)
                    nc.gpsimd.tensor_tensor(
                        lnu, lnu, rstd[:, i:i + 1].broadcast_to((128, D)),
                        op=ALU.mult)
                    vmid = scp.tile([128, D], F32, name="vmid", tag="sc")
                    nc.gpsimd.tensor_tensor(vmid, lnu, vN[:, tb, :], op=ALU.add)
                    nc.scalar.activation(sqf, vmid, ACTF.Square,
                                         accum_out=sq2s[:, i:i + 1])
                    vmids.append(vmid)
                veps2 = stgp.tile([128, 4], F32, name="cp_veps2", tag="stg")
                nc.gpsimd.tensor_scalar(veps2, sq2s, 1.0 / D, EPS, op0=ALU.mult,
                                        op1=ALU.add)
                sqv2 = stgp.tile([128, 4], F32, name="cp_sqv2", tag="stg")
                nc.scalar.sqrt(sqv2, veps2)
                rstd2 = stgp.tile([128, 4], F32, name="cp_rstd2", tag="stg")
                nc.vector.reciprocal(rstd2, sqv2)
                for i, tb in enumerate(tbs):
                    nc.gpsimd.tensor_tensor(
                        vN[:, tb, :], vmids[i],
                        rstd2[:, i:i + 1].broadcast_to((128, D)), op=ALU.mult)
                    nc.gpsimd.tensor_copy(vN_r[:, tb, :], vN[:, tb, :])

            def cphase_tr(g):
                # vT transposes for group g (emitted where inputs are ready)
                for tb in range(4 * g, 4 * g + 4):
                    for c in range(2):
                        tr128(vT[c][:, tb * 128:(tb + 1) * 128],
                              vN[:, tb, c * 128:(c + 1) * 128])

            def mlp(layer):
                # streamed over 8 weight-eighths; cphase chains emitted inside
                # the last eighth as each si group's update completes
                for q in range(NQ):
                    qs = slice(q * (N // NQ), (q + 1) * (N // NQ))
                    wxq = wq.tile([128, 2, N // NQ], F32R, name="wxq", tag="wxq")
                    nc.sync.dma_start(wxq[:], wx_r[:, :, qs])
                    wyq = wq.tile([128, 2, N // NQ], F32R, name="wyq", tag="wyq")
                    nc.sync.dma_start(wyq[:], wy_r[:, :, qs])
                    encq = wq.tile([128, NCHQ, D], F32R, name="encq", tag="encq")
                    nc.sync.dma_start(encq[:], enc_r[:, q * NCHQ:(q + 1) * NCHQ, :])
                    for si in range(NSUP):
                        sl = slice(si * TS, (si + 1) * TS)
                        psU = [ps256.tile([128, D], F32, name="psU", tag="ps256")
                               for _ in range(4)]
                        def psu_emit(ysb, nch):
                            for tb4 in range(4):
                                t4 = slice(tb4 * 128, (tb4 + 1) * 128)
                                nc.tensor.matmul(
                                    psU[tb4], ysb[:, t4], encq[:, nch, :],
                                    start=(nch == 0), stop=(nch == NCHQ - 1))
                        for nch in range(NCHQ):
                            psX = ps512.tile([128, TS], F32, name="psX", tag="ps512")
                            psY = ps512.tile([128, TS], F32, name="psY", tag="ps512")
                            ns = slice(nch * 128, (nch + 1) * 128)
                            for i, (wt, act) in enumerate(((wxq, vT), (wyq, lnaT))):
                                ps = psX if i == 0 else psY
                                for c in range(2):
                                    nc.tensor.matmul(ps, wt[:, c, ns], act[c][:, sl],
                                                     start=(c == 0), stop=(c == 1))
                            xr = blkp.tile([128, TS], F32, name="xr", tag="blk")
                            nc.scalar.activation(xr, psX, ACTF.Relu)
                            ysb = blkp.tile([128, TS], F32R, name="ysb", tag="blk")
                            nc.vector.scalar_tensor_tensor(
                                ysb, psY, 0.0, xr, op0=ALU.max, op1=ALU.mult)
                            psu_emit(ysb, nch)
                        if q == 0:
                            for tb4 in range(4):
                                copy_any(updA(si * 4 + tb4), psU[tb4])
                        elif q < NQ - 1:
                            # drain PSUM fast (ACT/DVE), accumulate on Pool
                            psUs = b4p.tile([128, 4, D], F32, name="psUs", tag="b4")
                            for tb4 in range(4):
                                copy_any(psUs[:, tb4, :], psU[tb4])
                            for tb4 in range(4):
                                tb = si * 4 + tb4
                                nc.gpsimd.tensor_tensor(updA(tb), psUs[:, tb4, :],
                                                        updA(tb), op=ALU.add)
                        else:
                            for tb4 in range(4):
                                tb = si * 4 + tb4
                                nc.vector.scalar_tensor_tensor(
                                    updA(tb), psU[tb4], 0.0, updA(tb), op0=ALU.add,
                                    op1=ALU.add, accum_out=sumsU[:, tb:tb + 1])
                        if q == NQ - 1:
                            cphase_chain(si)

            # ---------------- program ----------------
            for si in range(NSUP):
                rope_chunk(si)
            # attention staggered: drain right after each si (correct PSUM
            # recycling); stats/apply/transposes overlap the next si
            att_psum(0)
            att_drain(0)
            att_psum(1)
            att_drain(1)
            att_ln_rest(0)
            att_psum(2)
            att_drain(2)
            att_ln_rest(1)
            att_psum(3)
            att_drain(3)
            att_ln_rest(2)
            att_ln_rest(3)

            for layer in range(layers):
                mlp(layer)
                if layer < layers - 1:
                    cphase_tr(0)
                    rope_chunk(0)
                    cphase_tr(1)
                    rope_chunk(1)
                    att_psum(0)
                    att_drain(0)
                    cphase_tr(2)
                    rope_chunk(2)
                    cphase_tr(3)
                    rope_chunk(3)
                    att_psum(1)
                    att_drain(1)
                    att_ln_rest(0)
                    att_psum(2)
                    att_drain(2)
                    att_ln_rest(1)
                    att_psum(3)
                    att_drain(3)
                    att_ln_rest(2)
                    att_ln_rest(3)
                else:
                    for g in range(4):
                        cphase_tr(g)

            # ---------------- readout ----------------
            ro_s = blkp.tile([128, 2, D], F32R, name="ro_s", tag="blk")
            nc.sync.dma_start(ro_s[:], ro_r)
            for tb in range(NTB):
                psR = ps256.tile([128, D], F32, name="psR", tag="ps256")
                for c in range(2):
                    nc.tensor.matmul(psR, vT[c][:, tb * 128:(tb + 1) * 128],
                                     ro_s[:, c, :], start=(c == 0), stop=(c == 1))
                lo = scp.tile([128, VOCAB], F32, name="lo", tag="sc")
                copy_any(lo[:], psR[:])
                nc.sync.dma_start(out_d.ap()[tb * 128:(tb + 1) * 128, :], lo[:])

    nc.compile()
    return nc


_NC_CACHE = {}


def get_nc():
    if "nc" not in _NC_CACHE:
        _NC_CACHE["nc"] = build_nc()
    return _NC_CACHE["nc"]


def make_host_inputs(idx, wte, encoder, decoder_x, decoder_y, readout):
    idx = np.asarray(idx)
    wte = np.asarray(wte, dtype=np.float32)
    encoder = np.asarray(encoder, dtype=np.float32)
    decoder_x = np.asarray(decoder_x, dtype=np.float32)
    decoder_y = np.asarray(decoder_y, dtype=np.float32)
    readout = np.asarray(readout, dtype=np.float32)

    wx = decoder_x.transpose(1, 0, 2).reshape(D, N)
    wy = decoder_y.transpose(1, 0, 2).reshape(D, N)
    # partition-contiguous layouts for fast DMA: [p, c, n] with d = c*128 + p
    wx = np.ascontiguousarray(wx.reshape(2, 128, N).transpose(1, 0, 2))
    wy = np.ascontiguousarray(wy.reshape(2, 128, N).transpose(1, 0, 2))
    # enc: [p, o, d] with n = o*128 + p
    enc_s = np.ascontiguousarray(encoder.reshape(N // 128, 128, D).transpose(1, 0, 2))

    com = {"wx": wx, "wy": wy, "enc": enc_s}

    inv_freq = 1.0 / (10000.0 ** (np.arange(0, D, 2, dtype=np.float32) / D))  # [128]
    t = np.arange(T, dtype=np.float32)
    freqsT = inv_freq[:, None] * t[None, :]                   # [128, T]
    com["cosT"] = np.cos(freqsT).astype(np.float32)
    com["sinT"] = np.sin(freqsT).astype(np.float32)

    s_idx = np.arange(128, dtype=np.int32)[:, None]
    c_idx = np.arange(1024, dtype=np.int32)[None, :]
    com["maskbig"] = (s_idx <= c_idx - 384).astype(ml_dtypes.bfloat16)
    com["wte"] = wte
    com["ro"] = readout
    com["identm"] = np.eye(128, dtype=np.float32)

    in_maps = []
    for b in range(B):
        m = dict(com)
        m["idxf"] = idx[b].astype(np.float32).reshape(1, T)
        in_maps.append(m)
    return in_maps


def kernel(idx, wte, encoder, decoder_x, decoder_y, readout):
    nc = get_nc()
    in_maps = make_host_inputs(idx, wte, encoder, decoder_x, decoder_y, readout)
    res = bass_utils.run_bass_kernel_spmd(nc, in_maps, core_ids=list(range(B)))
    out = np.stack([res.results[b]["logits"] for b in range(B)], axis=0)
    return out.astype(np.float32)


# revision 41
# speedup vs baseline: 1.0143x; 1.0110x over previous
"""Trainium2 Bass kernel for the BDH dense-transformer problem.

Sharding: data-parallel over B=8 across the 8 NeuronCores (one batch
element per core, no collectives). Each core runs the full 6-layer
network on its [T=2048, D=256] slice.

Precision: all matmuls run single-pass float32r (1 cyc/row on PE for
output free size >= 256; HW-measured effective mantissa ~11 bits,
per-matmul rel err ~1.5e-4; 3-4x faster than fp32 / bf16x2-split
3-pass). Tensors feeding f32r matmuls are declared float32r so producer
instructions emit f32r-rounded outputs (BIR verifier requirement);
DMA-fed weights stay unrounded fp32 bits, which HW-measurably matches
DVE-rounded operands (the PE rounds operands internally). The residual
stream vN is kept in full fp32 -- f32r-rounding it every layer was the
dominant end-to-end error term (1.3e-2 -> 3.4e-3) -- with vN_r a
rounded copy consumed only by the attention accumulate matmul; the
update accumulator updS is dedicated fp32 (no partial-sum rounding).

Structure:
  - token embedding via one-hot matmul (iota + is_equal + PE)
  - v kept in both layouts: vT [D,T] (f32r) and vN [T,D] (fp32)
  - causal linear attention block-wise: energyT = qr@qr^T per
    [s128, t512] block (PSUM, diagonal blocks skip masked-out leading
    cols), bf16-mask multiply, aN accumulated in PSUM over s-chunks
  - MLP streamed over N in eighths (weights DMA'd per layer in
    host-pre-shuffled partition-contiguous layouts), relu(x)*relu(y)
    fused via ACT relu + DVE scalar_tensor_tensor, update accumulated
    in PSUM per eighth then drained into updS

Scheduling (engines are in-order; emission order = per-engine queue
order, so placement of every instruction matters):
  - one PSUM accumulation group per 2KB bank (start=True zeroes the
    whole bank): psU/psA get 4 banks (ps256), psX/psY/psE/pst 4 (ps512)
  - LayerNorm statistics batched 4 token-blocks at a time as [128,4]
    ops; variance computed uncentered (E[x^2] - mu^2), row sums fused
    into drain/accumulate ops (accum_out)
  - cphase v = ln(v + ln(update)) runs almost entirely on the otherwise
    idle Pool engine (applies via stride-0 broadcast tensor_tensor; the
    second LN needs no mean handling since v + ln(upd) has exactly zero
    row-mean), emitted inside the MLP's last weight-eighth so it
    overlaps remaining MLP work without delaying ACT/DVE (Pool cannot
    read PSUM or use pointer-scalar ops, hence the drain dance)
  - psU drained PSUM->SBUF via alternating ACT/DVE copies, accumulated
    into updS on Pool; frees banks fast at si transitions
  - attention LN: psA drained with fused row-sums right after each si
    (correct PSUM recycling), stats/apply on Pool, lnaT transposes
    emitted after the NEXT si's PSUM work so the in-order PE never
    stalls on them
  - vT transposes emitted after the whole MLP (inputs complete);
    group 2/3 behind next-layer attention si=0
  - rope chunked per 512 t-block, all on DVE (Pool is busy with
    cphase chains at the layer boundary where rope is latency-critical)
"""

import math

import numpy as np
import ml_dtypes

import concourse.bass as bass
import concourse.tile as tile
from concourse import bacc, mybir
from concourse import bass_utils

F32 = mybir.dt.float32
F32R = mybir.dt.float32r
BF16 = mybir.dt.bfloat16
I32 = mybir.dt.int32
ALU = mybir.AluOpType
ACTF = mybir.ActivationFunctionType
AXX = mybir.AxisListType.X

B, T, D, N, H, VOCAB, L = 8, 2048, 256, 8192, 4, 256, 6
EPS = 1e-5
TS = 512          # t-super width
NSUP = T // TS    # 4
NTB = T // 128    # 16
NQ = 8            # weight eighths along N
NCHQ = N // 128 // NQ  # 8 n-chunks per eighth


def build_nc(layers=L):
    nc = bacc.Bacc("TRN2", target_bir_lowering=False, debug=False)

    idx_d = nc.dram_tensor("idxf", [1, T], F32R, kind="ExternalInput")
    wte_d = nc.dram_tensor("wte", [VOCAB, D], F32R, kind="ExternalInput")
    wx_d = nc.dram_tensor("wx", [128, 2, N], F32R, kind="ExternalInput")
    wy_d = nc.dram_tensor("wy", [128, 2, N], F32R, kind="ExternalInput")
    enc_d = nc.dram_tensor("enc", [128, N // 128, D], F32R, kind="ExternalInput")
    ro_d = nc.dram_tensor("ro", [D, VOCAB], F32R, kind="ExternalInput")
    cos_d = nc.dram_tensor("cosT", [128, T], F32, kind="ExternalInput")
    sin_d = nc.dram_tensor("sinT", [128, T], F32, kind="ExternalInput")
    mask_d = nc.dram_tensor("maskbig", [128, 1024], BF16, kind="ExternalInput")
    ident_d = nc.dram_tensor("identm", [128, 128], F32, kind="ExternalInput")
    out_d = nc.dram_tensor("logits", [T, VOCAB], F32, kind="ExternalOutput")

    wx_r, wy_r, enc_r = wx_d.ap(), wy_d.ap(), enc_d.ap()
    wte_r = wte_d.ap().rearrange("(c p) d -> p c d", p=128)
    ro_r = ro_d.ap().rearrange("(c p) d -> p c d", p=128)

    with tile.TileContext(nc) as tc:
        with tc.tile_pool(name="persist", bufs=1) as pp, \
             tc.tile_pool(name="wq", bufs=2) as wq, \
             tc.tile_pool(name="blk", bufs=6) as blkp, \
             tc.tile_pool(name="sc", bufs=12) as scp, \
             tc.tile_pool(name="st", bufs=32) as stp, \
             tc.tile_pool(name="stg", bufs=24) as stgp, \
             tc.tile_pool(name="b4", bufs=4) as b4p, \
             tc.tile_pool(name="ps512", bufs=4, space="PSUM") as ps512, \
             tc.tile_pool(name="ps256", bufs=4, space="PSUM") as ps256:

            vT = [pp.tile([128, T], F32R, name=f"vT{c}", tag=f"vT{c}") for c in range(2)]
            # vN holds the residual stream in full fp32; vN_r is its
            # f32r-rounded copy for the attention accumulate matmul.
            vN = pp.tile([128, NTB, D], F32, name="vN", tag="vN")
            vN_r = pp.tile([128, NTB, D], F32R, name="vN_r", tag="vN_r")
            qrT = [pp.tile([128, T], F32R, name=f"qrT{c}", tag=f"qrT{c}") for c in range(2)]
            lnaT = [pp.tile([128, T], F32R, name=f"lnaT{c}", tag=f"lnaT{c}") for c in range(2)]
            updS = pp.tile([128, NTB, D], F32, name="updS", tag="updS")
            sumsU = pp.tile([128, NTB], F32, name="sumsU", tag="sumsU")

            def updA(tb):
                return updS[:, tb, :]
            cosT = pp.tile([128, T], F32, name="cosT", tag="cosT")
            sinT = pp.tile([128, T], F32, name="sinT", tag="sinT")
            maskb = pp.tile([128, 1024], BF16, name="maskb", tag="maskb")

            ident = pp.tile([128, 128], F32, name="ident", tag="ident")
            iota_f = pp.tile([128, 2], F32, name="iota_f", tag="iota_f")

            nc.sync.dma_start(cosT[:], cos_d.ap())
            nc.sync.dma_start(sinT[:], sin_d.ap())
            nc.sync.dma_start(maskb[:], mask_d.ap())
            nc.sync.dma_start(ident[:], ident_d.ap())

            copy_flip = [0]

            def copy_any(dst, src):
                # alternate PSUM->SBUF copies between ACT and DVE
                # (Pool/GPSIMD cannot access PSUM)
                copy_flip[0] ^= 1
                if copy_flip[0]:
                    nc.scalar.copy(dst, src)
                else:
                    nc.vector.tensor_copy(dst, src)

            def tr128(dst, src):
                pst = ps512.tile([128, 512], F32, name="pst", tag="ps512")
                if src.dtype != F32:
                    src = src.bitcast(F32)
                nc.tensor.transpose(pst[:, :128], src, ident[:])
                copy_any(dst, pst[:, :128])

            def ln_nat(src, dst, sums=None):
                """Single-block LayerNorm (embedding only)."""
                if sums is None:
                    sums = stp.tile([128, 1], F32, name="s1", tag="st")
                    nc.vector.reduce_sum(sums, src, axis=AXX)
                negmean = stp.tile([128, 1], F32, name="negmean", tag="st")
                nc.vector.tensor_scalar_mul(negmean, sums, -1.0 / D)
                sq = scp.tile([128, D], F32, name="sq", tag="sc")
                sqs = stp.tile([128, 1], F32, name="sqs", tag="st")
                nc.scalar.activation(sq, src, ACTF.Square, bias=negmean, scale=1.0,
                                     accum_out=sqs)
                veps = stp.tile([128, 1], F32, name="veps", tag="st")
                nc.vector.tensor_scalar(veps, sqs, 1.0 / D, EPS, op0=ALU.mult, op1=ALU.add)
                sqv = stp.tile([128, 1], F32, name="sqv", tag="st")
                nc.scalar.sqrt(sqv, veps)
                rstd = stp.tile([128, 1], F32, name="rstd", tag="st")
                nc.vector.reciprocal(rstd, sqv)
                negmurs = stp.tile([128, 1], F32, name="negmurs", tag="st")
                nc.vector.tensor_tensor(negmurs, negmean, rstd, op=ALU.mult)
                nc.scalar.activation(dst, src, ACTF.Identity, bias=negmurs, scale=rstd)

            # ---------------- embedding: v = ln(wte[idx]) ----------------
            iota_i = pp.tile([128, 2], I32, name="iota_i", tag="iota_i")
            for c in range(2):
                nc.gpsimd.iota(iota_i[:, c:c + 1], pattern=[[1, 1]], base=c * 128,
                               channel_multiplier=1)
            nc.vector.tensor_copy(iota_f[:], iota_i[:])
            idx_b = lnaT[0]  # scratch alias
            nc.sync.dma_start(idx_b[:], idx_d.ap().partition_broadcast(128))
            for c in range(2):
                # one-hot^T chunk in qrT[c] (scratch alias); 0/1 exact in f32r
                nc.vector.tensor_scalar(qrT[c][:], idx_b[:], iota_f[:, c:c + 1], None,
                                        op0=ALU.is_equal)
            wte_s = blkp.tile([128, 2, D], F32R, name="wte_s", tag="blk")
            nc.sync.dma_start(wte_s[:], wte_r)
            for tb in range(NTB):
                psA = ps256.tile([128, D], F32, name="psE", tag="ps256")
                for c in range(2):
                    nc.tensor.matmul(psA, qrT[c][:, tb * 128:(tb + 1) * 128],
                                     wte_s[:, c, :], start=(c == 0), stop=(c == 1))
                ln_nat(psA, vN[:, tb, :])
                nc.gpsimd.tensor_copy(vN_r[:, tb, :], vN[:, tb, :])
                for c in range(2):
                    tr128(vT[c][:, tb * 128:(tb + 1) * 128], vN[:, tb, c * 128:(c + 1) * 128])

            # ---------------- building blocks ----------------
            rsc = lnaT[1]  # rope scratch (dead region at rope time)

            def rope_chunk(si):
                # qrT[:, si block] = vT*cos +/- rot*sin, split DVE/Pool
                sl = slice(si * TS, (si + 1) * TS)
                e0 = nc.vector if si % 2 == 0 else nc.gpsimd
                e1 = nc.gpsimd if si % 2 == 0 else nc.vector
                e0.tensor_tensor(qrT[0][:, sl], vT[0][:, sl], cosT[:, sl], op=ALU.mult)
                e1.tensor_tensor(rsc[:, sl], vT[1][:, sl], sinT[:, sl], op=ALU.mult)
                e0.tensor_tensor(qrT[0][:, sl], qrT[0][:, sl], rsc[:, sl], op=ALU.subtract)
                e1.tensor_tensor(qrT[1][:, sl], vT[1][:, sl], cosT[:, sl], op=ALU.mult)
                e0.tensor_tensor(rsc[:, sl], vT[0][:, sl], sinT[:, sl], op=ALU.mult)
                e1.tensor_tensor(qrT[1][:, sl], qrT[1][:, sl], rsc[:, sl], op=ALU.add)

            att_state = {}

            def att_psum(si):
                # energy blocks + psA accumulation for one si super-block
                psA = [ps256.tile([128, D], F32, name="psA", tag="ps256")
                       for _ in range(4)]

                def psa_emit(eT, sc):
                    for tb4 in range(4):
                        tb = si * 4 + tb4
                        if sc <= tb:
                            nc.tensor.matmul(psA[tb4],
                                             eT[:, tb4 * 128:(tb4 + 1) * 128],
                                             vN_r[:, sc, :], start=(sc == 0),
                                             stop=(sc == tb))
                for sc in range(4 * si + 4):
                    k = sc - 4 * si
                    off = max(k, 0) * 128  # masked-out leading cols skipped
                    psE = ps512.tile([128, TS], F32, name="psE", tag="ps512")
                    for c in range(2):
                        nc.tensor.matmul(psE[:, off:],
                                         qrT[c][:, sc * 128:(sc + 1) * 128],
                                         qrT[c][:, si * TS + off:(si + 1) * TS],
                                         start=(c == 0), stop=(c == 1))
                    eT = blkp.tile([128, TS], F32R, name="eT", tag="blk")
                    if k < 0:
                        nc.scalar.copy(eT[:], psE[:])
                    else:
                        nc.vector.tensor_tensor(
                            eT[:, off:], psE[:, off:], maskb[:, 384: 896 - off],
                            op=ALU.mult)
                    psa_emit(eT, sc)
                att_state[si] = psA

            def att_drain(si):
                # drain psA to SBUF with fused row-sums (ACT/DVE). Emitted
                # immediately after att_psum(si) so the PSUM banks recycle
                # with correct dependencies before the next si allocates them.
                psAb = att_state.pop(si)
                psAs = b4p.tile([128, 4, D], F32, name="psAs", tag="b4")
                sums = stgp.tile([128, 4], F32, name="at_sums", tag="stg")
                for j in range(4):
                    src = psAb[j][:]
                    if j % 2 == 0:
                        nc.scalar.activation(psAs[:, j, :], src, ACTF.Identity,
                                             accum_out=sums[:, j:j + 1])
                    else:
                        nc.vector.tensor_scalar(psAs[:, j, :], src, 0.0, 0.0,
                                                op0=ALU.add, op1=ALU.add,
                                                accum_out=sums[:, j:j + 1])
                att_state[(si, 'drained')] = (psAs, sums)

            def att_ln_rest(si):
                # stats on ACT/Pool, apply via Pool broadcast-tt, transposes
                psAs, sums = att_state.pop((si, 'drained'))
                sqs = stgp.tile([128, 4], F32, name="at_sqs", tag="stg")
                sqf = scp.tile([128, D], F32, name="at_scr", tag="sc")
                for j in range(4):
                    nc.scalar.activation(sqf, psAs[:, j, :], ACTF.Square,
                                         accum_out=sqs[:, j:j + 1])
                negmean = stgp.tile([128, 4], F32, name="at_nm", tag="stg")
                nc.gpsimd.tensor_scalar(negmean, sums, -1.0 / D, 0.0,
                                        op0=ALU.mult, op1=ALU.add)
                msq = stgp.tile([128, 4], F32, name="at_msq", tag="stg")
                nc.gpsimd.tensor_tensor(msq, negmean, negmean, op=ALU.mult)
                ex2 = stgp.tile([128, 4], F32, name="at_ex2", tag="stg")
                nc.gpsimd.tensor_scalar(ex2, sqs, 1.0 / D, EPS, op0=ALU.mult,
                                        op1=ALU.add)
                veps = stgp.tile([128, 4], F32, name="at_veps", tag="stg")
                nc.gpsimd.tensor_tensor(veps, ex2, msq, op=ALU.subtract)
                sqv = stgp.tile([128, 4], F32, name="at_sqv", tag="stg")
                nc.scalar.sqrt(sqv, veps)
                rstd = stgp.tile([128, 4], F32, name="at_rstd", tag="stg")
                nc.vector.reciprocal(rstd, sqv)
                lnas = []
                for j in range(4):
                    lna = scp.tile([128, D], F32, name="lna_n", tag="sc")
                    nc.gpsimd.tensor_tensor(
                        lna, psAs[:, j, :],
                        negmean[:, j:j + 1].broadcast_to((128, D)), op=ALU.add)
                    nc.gpsimd.tensor_tensor(
                        lna, lna, rstd[:, j:j + 1].broadcast_to((128, D)),
                        op=ALU.mult)
                    lnas.append(lna)
                for tb4 in range(4):
                    tb = si * 4 + tb4
                    for c in range(2):
                        tr128(lnaT[c][:, tb * 128:(tb + 1) * 128],
                              lnas[tb4][:, c * 128:(c + 1) * 128])

            def cphase_chain(g):
                # v = ln(v + ln(update)) values for token-blocks 4g..4g+3,
                # entirely on Pool (plus tiny ACT sqrt / DVE reciprocal).
                # Uncentered variance; the second LN exploits that
                # v + ln(upd) has exactly zero row-mean.
                tbs = list(range(4 * g, 4 * g + 4))
                sqs = stgp.tile([128, 4], F32, name="cp_sqs", tag="stg")
                sqf = scp.tile([128, D], F32, name="cp_scr", tag="sc")
                for i, tb in enumerate(tbs):
                    nc.scalar.activation(sqf, updA(tb), ACTF.Square,
                                         accum_out=sqs[:, i:i + 1])
                negmean = stgp.tile([128, 4], F32, name="cp_nm", tag="stg")
                nc.gpsimd.tensor_scalar(negmean, sumsU[:, 4 * g:4 * g + 4],
                                        -1.0 / D, 0.0, op0=ALU.mult, op1=ALU.add)
                msq = stgp.tile([128, 4], F32, name="cp_msq", tag="stg")
                nc.gpsimd.tensor_tensor(msq, negmean, negmean, op=ALU.mult)
                ex2 = stgp.tile([128, 4], F32, name="cp_ex2", tag="stg")
                nc.gpsimd.tensor_scalar(ex2, sqs, 1.0 / D, EPS, op0=ALU.mult,
                                        op1=ALU.add)
                veps = stgp.tile([128, 4], F32, name="cp_veps", tag="stg")
                nc.gpsimd.tensor_tensor(veps, ex2, msq, op=ALU.subtract)
                sqv = stgp.tile([128, 4], F32, name="cp_sqv", tag="stg")
                nc.scalar.sqrt(sqv, veps)
                rstd = stgp.tile([128, 4], F32, name="cp_rstd", tag="stg")
                nc.vector.reciprocal(rstd, sqv)
                sq2s = stgp.tile([128, 4], F32, name="cp_sq2s", tag="stg")
                vmids = []
                for i, tb in enumerate(tbs):
                    lnu = scp.tile([128, D], F32, name="lnu", tag="sc")
                    nc.gpsimd.tensor_tensor(
                        lnu, updA(tb),
                        negmean[:, i:i + 1].broadcast_to((128, D)), op=ALU.add)
                    nc.gpsimd.tensor_tensor(
                        lnu, lnu, rstd[:, i:i + 1].broadcast_to((128, D)),
                        op=ALU.mult)
                    vmid = scp.tile([128, D], F32, name="vmid", tag="sc")
                    nc.gpsimd.tensor_tensor(vmid, lnu, vN[:, tb, :], op=ALU.add)
                    nc.scalar.activation(sqf, vmid, ACTF.Square,
                                         accum_out=sq2s[:, i:i + 1])
                    vmids.append(vmid)
                veps2 = stgp.tile([128, 4], F32, name="cp_veps2", tag="stg")
                nc.gpsimd.tensor_scalar(veps2, sq2s, 1.0 / D, EPS, op0=ALU.mult,
                                        op1=ALU.add)
                sqv2 = stgp.tile([128, 4], F32, name="cp_sqv2", tag="stg")
                nc.scalar.sqrt(sqv2, veps2)
                rstd2 = stgp.tile([128, 4], F32, name="cp_rstd2", tag="stg")
                nc.vector.reciprocal(rstd2, sqv2)
                for i, tb in enumerate(tbs):
                    nc.gpsimd.tensor_tensor(
                        vN[:, tb, :], vmids[i],
                        rstd2[:, i:i + 1].broadcast_to((128, D)), op=ALU.mult)
                    nc.gpsimd.tensor_copy(vN_r[:, tb, :], vN[:, tb, :])

            def cphase_tr(g):
                # vT transposes for group g (emitted where inputs are ready)
                for tb in range(4 * g, 4 * g + 4):
                    for c in range(2):
                        tr128(vT[c][:, tb * 128:(tb + 1) * 128],
                              vN[:, tb, c * 128:(c + 1) * 128])

            def mlp(layer):
                # streamed over 8 weight-eighths; cphase chains emitted inside
                # the last eighth as each si group's update completes
                for q in range(NQ):
                    qs = slice(q * (N // NQ), (q + 1) * (N // NQ))
                    wxq = wq.tile([128, 2, N // NQ], F32R, name="wxq", tag="wxq")
                    nc.sync.dma_start(wxq[:], wx_r[:, :, qs])
                    wyq = wq.tile([128, 2, N // NQ], F32R, name="wyq", tag="wyq")
                    nc.sync.dma_start(wyq[:], wy_r[:, :, qs])
                    encq = wq.tile([128, NCHQ, D], F32R, name="encq", tag="encq")
                    nc.sync.dma_start(encq[:], enc_r[:, q * NCHQ:(q + 1) * NCHQ, :])
                    for si in range(NSUP):
                        sl = slice(si * TS, (si + 1) * TS)
                        psU = [ps256.tile([128, D], F32, name="psU", tag="ps256")
                               for _ in range(4)]
                        def psu_emit(ysb, nch):
                            for tb4 in range(4):
                                t4 = slice(tb4 * 128, (tb4 + 1) * 128)
                                nc.tensor.matmul(
                                    psU[tb4], ysb[:, t4], encq[:, nch, :],
                                    start=(nch == 0), stop=(nch == NCHQ - 1))
                        for nch in range(NCHQ):
                            psX = ps512.tile([128, TS], F32, name="psX", tag="ps512")
                            psY = ps512.tile([128, TS], F32, name="psY", tag="ps512")
                            ns = slice(nch * 128, (nch + 1) * 128)
                            for i, (wt, act) in enumerate(((wxq, vT), (wyq, lnaT))):
                                ps = psX if i == 0 else psY
                                for c in range(2):
                                    nc.tensor.matmul(ps, wt[:, c, ns], act[c][:, sl],
                                                     start=(c == 0), stop=(c == 1))
                            xr = blkp.tile([128, TS], F32, name="xr", tag="blk")
                            nc.scalar.activation(xr, psX, ACTF.Relu)
                            ysb = blkp.tile([128, TS], F32R, name="ysb", tag="blk")
                            nc.vector.scalar_tensor_tensor(
                                ysb, psY, 0.0, xr, op0=ALU.max, op1=ALU.mult)
                            psu_emit(ysb, nch)
                        if q == 0:
                            for tb4 in range(4):
                                copy_any(updA(si * 4 + tb4), psU[tb4])
                        elif q < NQ - 1:
                            # drain PSUM fast (ACT/DVE), accumulate on Pool
                            psUs = b4p.tile([128, 4, D], F32, name="psUs", tag="b4")
                            for tb4 in range(4):
                                copy_any(psUs[:, tb4, :], psU[tb4])
                            for tb4 in range(4):
                                tb = si * 4 + tb4
                                nc.gpsimd.tensor_tensor(updA(tb), psUs[:, tb4, :],
                                                        updA(tb), op=ALU.add)
                        else:
                            for tb4 in range(4):
                                tb = si * 4 + tb4
                                nc.vector.scalar_tensor_tensor(
                                    updA(tb), psU[tb4], 0.0, updA(tb), op0=ALU.add,
                                    op1=ALU.add, accum_out=sumsU[:, tb:tb + 1])
                        if q == NQ - 1:
                            cphase_chain(si)

            # ---------------- program ----------------
            for si in range(NSUP):
                rope_chunk(si)
            # attention staggered: drain right after each si (correct PSUM
            # recycling); stats/apply/transposes overlap the next si
            att_psum(0)
            att_drain(0)
            att_psum(1)
            att_drain(1)
            att_ln_rest(0)
            att_psum(2)
            att_drain(2)
            att_ln_rest(1)
            att_psum(3)
            att_drain(3)
            att_ln_rest(2)
            att_ln_rest(3)

            for layer in range(layers):
                mlp(layer)
                if layer < layers - 1:
                    cphase_tr(0)
                    rope_chunk(0)
                    cphase_tr(1)
                    att_psum(0)
                    att_drain(0)
                    rope_chunk(1)
                    cphase_tr(2)
                    rope_chunk(2)
                    cphase_tr(3)
                    rope_chunk(3)
                    att_psum(1)
                    att_drain(1)
                    att_ln_rest(0)
                    att_psum(2)
                    att_drain(2)
                    att_ln_rest(1)
                    att_psum(3)
                    att_drain(3)
                    att_ln_rest(2)
                    att_ln_rest(3)
                else:
                    for g in range(4):
                        cphase_tr(g)

            # ---------------- readout ----------------
            ro_s = blkp.tile([128, 2, D], F32R, name="ro_s", tag="blk")
            nc.sync.dma_start(ro_s[:], ro_r)
            for tb in range(NTB):
                psR = ps256.tile([128, D], F32, name="psR", tag="ps256")
                for c in range(2):
                    nc.tensor.matmul(psR, vT[c][:, tb * 128:(tb + 1) * 128],
                                     ro_s[:, c, :], start=(c == 0), stop=(c == 1))
                lo = scp.tile([128, VOCAB], F32, name="lo", tag="sc")
                copy_any(lo[:], psR[:])
                nc.sync.dma_start(out_d.ap()[tb * 128:(tb + 1) * 128, :], lo[:])

    nc.compile()
    return nc


_NC_CACHE = {}


def get_nc():
    if "nc" not in _NC_CACHE:
        _NC_CACHE["nc"] = build_nc()
    return _NC_CACHE["nc"]


def make_host_inputs(idx, wte, encoder, decoder_x, decoder_y, readout):
    idx = np.asarray(idx)
    wte = np.asarray(wte, dtype=np.float32)
    encoder = np.asarray(encoder, dtype=np.float32)
    decoder_x = np.asarray(decoder_x, dtype=np.float32)
    decoder_y = np.asarray(decoder_y, dtype=np.float32)
    readout = np.asarray(readout, dtype=np.float32)

    wx = decoder_x.transpose(1, 0, 2).reshape(D, N)
    wy = decoder_y.transpose(1, 0, 2).reshape(D, N)
    # partition-contiguous layouts for fast DMA: [p, c, n] with d = c*128 + p
    wx = np.ascontiguousarray(wx.reshape(2, 128, N).transpose(1, 0, 2))
    wy = np.ascontiguousarray(wy.reshape(2, 128, N).transpose(1, 0, 2))
    # enc: [p, o, d] with n = o*128 + p
    enc_s = np.ascontiguousarray(encoder.reshape(N // 128, 128, D).transpose(1, 0, 2))

    com = {"wx": wx, "wy": wy, "enc": enc_s}

    inv_freq = 1.0 / (10000.0 ** (np.arange(0, D, 2, dtype=np.float32) / D))  # [128]
    t = np.arange(T, dtype=np.float32)
    freqsT = inv_freq[:, None] * t[None, :]                   # [128, T]
    com["cosT"] = np.cos(freqsT).astype(np.float32)
    com["sinT"] = np.sin(freqsT).astype(np.float32)

    s_idx = np.arange(128, dtype=np.int32)[:, None]
    c_idx = np.arange(1024, dtype=np.int32)[None, :]
    com["maskbig"] = (s_idx <= c_idx - 384).astype(ml_dtypes.bfloat16)
    com["wte"] = wte
    com["ro"] = readout
    com["identm"] = np.eye(128, dtype=np.float32)

    in_maps = []
    for b in range(B):
        m = dict(com)
        m["idxf"] = idx[b].astype(np.float32).reshape(1, T)
        in_maps.append(m)
    return in_maps


def kernel(idx, wte, encoder, decoder_x, decoder_y, readout):
    nc = get_nc()
    in_maps = make_host_inputs(idx, wte, encoder, decoder_x, decoder_y, readout)
    res = bass_utils.run_bass_kernel_spmd(nc, in_maps, core_ids=list(range(B)))
    out = np.stack([res.results[b]["logits"] for b in range(B)], axis=0)
    return out.astype(np.float32)
